# revision 13
# baseline (speedup 1.0000x reference)
"""Complex CNN 2d (conv + complex-combine + training-mode BatchNorm) on 8 trn2 cores.

Strategy (hardcoded for B=32, Cin=2, Cout=64, H=W=128, K=5, pad=2, stride=1):
  - Data-parallel over batch: 4 images per core.
  - Conv as matmul: contract dim = (plane, ky) = 4*5 = 20 rows; kx handled by
    free-dim AP shift with 5-step PSUM accumulation; the 4 local images run as
    4 concurrent 32-row-group matmuls (tile_position via base_partition).
  - Input is host-prepacked into a [128, 128, 132] SBUF image: partition
    (32*b + 5*plane + ky) holds the plane shifted down by ky and padded
    horizontally.  Out channels = 128 = [64 real | 64 imag] (complex combine
    is folded into the weight matrix signs).
  - Exact global BN stats: pass 1 computes conv + per-core (mean, E[Y^2]) via
    bn_stats/bn_aggr, AllReduce over the 8 cores, then pass 2 recomputes conv
    and applies y*scale+shift (Y never round-trips to HBM).
  - Conv bias br/bi provably cancels in BN (shifts mean equally) -> ignored.
"""

import sys

sys.path.insert(0, "/opt/trn_rl_repo")

import numpy as np

B, CIN, COUT, H, W, K, PAD = 32, 2, 64, 128, 128, 5, 2
EPS = 1e-5
NCORES = 8
BL = B // NCORES  # 4 local images per core
NPLANES = 2 * CIN  # r0, r1, i0, i1
KROWS = NPLANES * K  # 20 contract rows per image
WPAD = W + K - 1  # 132
CTOT = 2 * COUT  # 128 fused out channels: [real 64 | imag 64]
YB = 4  # y-rows per PSUM bank (4*128 = 512 = one fp32 bank)
NBLK = H // YB  # 32 blocks
MM_DT = "float32r"  # matmul streaming dtype (float32r: full-rate fp32 path)

_CACHE = {}


def _build_nc():
    import concourse.tile as tile
    from concourse import bacc, mybir

    f32 = mybir.dt.float32
    mdt = getattr(mybir.dt, MM_DT)

    # Bacc (not plain Bass): its compile pipeline splits multi-sem waits into
    # event-semaphore preludes, which TRN2 instruction structs require
    nc = bacc.Bacc(num_devices=NCORES)
    # image planes and conv weights packed in one tensor so each 32-partition
    # row group arrives via a single DMA (a fused-LDW matmul only supports one
    # sync wait)
    zwlen = H * WPAD + K * CTOT
    z_d = nc.dram_tensor("zw", [128, zwlen], mdt, kind="ExternalInput")
    g_d = nc.dram_tensor("gamma", [CTOT, 1], f32, kind="ExternalInput")
    bt_d = nc.dram_tensor("beta", [CTOT, 1], f32, kind="ExternalInput")
    o_d = nc.dram_tensor("out", [CTOT, BL, H, W], f32, kind="ExternalOutput")

    with tile.TileContext(nc) as tc:
        with (
            tc.tile_pool(name="const", bufs=1) as const,
            tc.tile_pool(name="psum", bufs=8, space="PSUM") as psum,
            tc.tile_pool(name="outp", bufs=8) as outp,
            tc.tile_pool(name="small", bufs=1) as small,
            tc.tile_pool(name="dram", bufs=1, space="DRAM") as dram,
        ):
            zw = const.tile([128, zwlen], mdt)
            # one DMA per 32-partition row group so each matmul waits on one sem
            for c in range(4):
                nc.sync.dma_start(
                    out=zw[32 * c : 32 * c + 32], in_=z_d[32 * c : 32 * c + 32]
                )
            zt = zw[:, : H * WPAD].rearrange("p (h w) -> p h w", h=H)
            wt = zw[:, H * WPAD :].rearrange("p (k c) -> p k c", k=K)
            gt = const.tile([CTOT, 1], f32)
            nc.sync.dma_start(out=gt[:], in_=g_d[:])
            bt = const.tile([CTOT, 1], f32)
            nc.sync.dma_start(out=bt[:], in_=bt_d[:])
            eps_t = const.tile([CTOT, 1], f32)
            nc.vector.memset(eps_t[:], EPS)

            # 8 persistent PSUM bank tiles: same tensors across all blocks, so
            # bank-WAW between matmuls is same-tensor/same-engine (program
            # order, no sem) and each first matmul carries at most the single
            # WAR wait on the previous consumer.  A fused-LDW (fp32r) matmul
            # only supports one sync wait in walrus codegen.
            pbanks = [
                psum.tile([CTOT, YB, W], f32, name=f"pbank{i}", tag=f"pbank{i}", bufs=1)
                for i in range(2 * BL)
            ]

            def conv_block(blk, consume):
                ys = blk * YB
                banks = pbanks[BL * (blk % 2) : BL * (blk % 2) + BL]
                for kxi in range(K):
                    for b in range(BL):
                        nc.tensor.matmul(
                            banks[b][:, :, :],
                            wt[32 * b : 32 * b + KROWS, kxi, :],
                            zt[32 * b : 32 * b + KROWS, ys : ys + YB, kxi : kxi + W],
                            start=(kxi == 0),
                            stop=(kxi == K - 1),
                            tile_position=(32 * b, 0),
                        )
                for b in range(BL):
                    consume(b, banks[b], ys)

            # ---- pass 1: conv + per-core stats ----
            stats = small.tile([CTOT, NBLK * BL, 6], f32)

            def stat_consume(b, bank, ys):
                blk = ys // YB
                e = blk * BL + b
                nc.vector.bn_stats(
                    out=stats[:, e, :],
                    in_=bank[:, :, :].rearrange("p a b -> p (a b)"),
                )

            for blk in range(NBLK):
                conv_block(blk, stat_consume)

            mv = small.tile([CTOT, 2], f32)
            nc.vector.bn_aggr(out=mv[:], in_=stats[:])
            # pack (mean, E[Y^2]) for the cross-core all-reduce
            pair = small.tile([CTOT, 2], f32)
            nc.vector.tensor_copy(out=pair[:, 0:1], in_=mv[:, 0:1])
            msq = small.tile([CTOT, 1], f32)
            nc.vector.tensor_mul(out=msq[:], in0=mv[:, 0:1], in1=mv[:, 0:1])
            nc.vector.tensor_add(out=pair[:, 1:2], in0=mv[:, 1:2], in1=msq[:])

            cc_in = dram.tile([CTOT, 2], f32)
            cc_out = dram.tile([CTOT, 2], f32)
            nc.gpsimd.dma_start(out=cc_in[:], in_=pair[:])
            nc.gpsimd.collective_compute(
                "AllReduce",
                mybir.AluOpType.add,
                replica_groups=[list(range(NCORES))],
                ins=[cc_in[:].opt()],
                outs=[cc_out[:].opt()],
            )
            red = small.tile([CTOT, 2], f32)
            nc.gpsimd.dma_start(out=red[:], in_=cc_out[:])

            # global mean / var -> scale, shift
            mean_g = small.tile([CTOT, 1], f32)
            nc.vector.tensor_scalar_mul(out=mean_g[:], in0=red[:, 0:1], scalar1=1.0 / NCORES)
            ey2_g = small.tile([CTOT, 1], f32)
            nc.vector.tensor_scalar_mul(out=ey2_g[:], in0=red[:, 1:2], scalar1=1.0 / NCORES)
            mg2 = small.tile([CTOT, 1], f32)
            nc.vector.tensor_mul(out=mg2[:], in0=mean_g[:], in1=mean_g[:])
            var_g = small.tile([CTOT, 1], f32)
            nc.vector.tensor_sub(out=var_g[:], in0=ey2_g[:], in1=mg2[:])
            std = small.tile([CTOT, 1], f32)
            nc.scalar.activation(
                out=std[:], in_=var_g[:],
                func=mybir.ActivationFunctionType.Sqrt,
                bias=eps_t[:], scale=1.0,
            )
            rstd = small.tile([CTOT, 1], f32)
            nc.vector.reciprocal(out=rstd[:], in_=std[:])
            scale_t = small.tile([CTOT, 1], f32)
            nc.vector.tensor_mul(out=scale_t[:], in0=gt[:], in1=rstd[:])
            mscale = small.tile([CTOT, 1], f32)
            nc.vector.tensor_mul(out=mscale[:], in0=mean_g[:], in1=scale_t[:])
            shift_t = small.tile([CTOT, 1], f32)
            nc.vector.tensor_sub(out=shift_t[:], in0=bt[:], in1=mscale[:])

            # ---- pass 2: conv again + affine apply + store ----
            def apply_consume(b, bank, ys):
                ob = outp.tile([CTOT, YB, W], f32, tag="ob")
                blk = ys // YB
                if (blk * BL + b) % 2 == 0:
                    nc.vector.tensor_scalar(
                        out=ob[:], in0=bank[:, :, :],
                        scalar1=scale_t[:], scalar2=shift_t[:],
                        op0=mybir.AluOpType.mult, op1=mybir.AluOpType.add,
                    )
                else:
                    nc.scalar.activation(
                        out=ob[:], in_=bank[:, :, :],
                        func=mybir.ActivationFunctionType.Identity,
                        bias=shift_t[:], scale=scale_t[:],
                    )
                nc.sync.dma_start(out=o_d[:, b, ys : ys + YB, :], in_=ob[:])

            for blk in range(NBLK):
                conv_block(blk, apply_consume)

    nc.finalize()
    return nc


def _get_nc():
    if "nc" not in _CACHE:
        _CACHE["nc"] = _build_nc()
    return _CACHE["nc"]


def _pack_inputs(Xr, Xi, Wr, Wi, gamma_r, beta_r, gamma_i, beta_i):
    planes = np.stack([Xr[:, 0], Xr[:, 1], Xi[:, 0], Xi[:, 1]], axis=1)  # [B,4,H,W]
    planes = np.ascontiguousarray(planes, dtype=np.float32)
    Z = np.zeros((NCORES, 128, H, WPAD), np.float32)
    for ky in range(K):
        r0, r1 = max(0, PAD - ky), min(H, H + PAD - ky)
        s0, s1 = r0 + ky - PAD, r1 + ky - PAD
        for bl in range(BL):
            for pi in range(NPLANES):
                p = 32 * bl + 5 * pi + ky
                for c in range(NCORES):
                    Z[c, p, r0:r1, PAD : PAD + W] = planes[BL * c + bl, pi, s0:s1, :]

    Wf = np.zeros((128, K, CTOT), np.float32)
    for pi in range(NPLANES):
        for ky in range(K):
            row = 5 * pi + ky
            if pi < 2:
                Wf[row, :, :COUT] = Wr[:, pi, ky, :].T
                Wf[row, :, COUT:] = Wi[:, pi, ky, :].T
            else:
                Wf[row, :, :COUT] = -Wi[:, pi - 2, ky, :].T
                Wf[row, :, COUT:] = Wr[:, pi - 2, ky, :].T
    Wrep = np.zeros((128, K, CTOT), np.float32)
    for b in range(BL):
        Wrep[32 * b : 32 * b + KROWS] = Wf[:KROWS]

    gam = np.concatenate([gamma_r, gamma_i]).astype(np.float32).reshape(CTOT, 1)
    bet = np.concatenate([beta_r, beta_i]).astype(np.float32).reshape(CTOT, 1)

    zwlen = H * WPAD + K * CTOT
    ZW = np.zeros((NCORES, 128, zwlen), np.float32)
    ZW[:, :, : H * WPAD] = Z.reshape(NCORES, 128, H * WPAD)
    ZW[:, :, H * WPAD :] = Wrep.reshape(128, K * CTOT)[None]

    in_maps = [{"zw": ZW[c], "gamma": gam, "beta": bet} for c in range(NCORES)]
    return in_maps


def _run(in_maps, trace=False):
    from concourse.bass_utils import run_bass_kernel_spmd

    nc = _get_nc()
    return run_bass_kernel_spmd(nc, in_maps, list(range(NCORES)), trace=trace)


def kernel(Xr, Xi, Wr, Wi, br, bi, gamma_r, beta_r, gamma_i, beta_i, _trace=False):
    Xr = np.asarray(Xr, np.float32)
    Xi = np.asarray(Xi, np.float32)
    Wr = np.asarray(Wr, np.float32)
    Wi = np.asarray(Wi, np.float32)
    in_maps = _pack_inputs(
        Xr, Xi, Wr, Wi,
        np.asarray(gamma_r), np.asarray(beta_r),
        np.asarray(gamma_i), np.asarray(beta_i),
    )
    res = _run(in_maps, trace=_trace)
    out = np.empty((2, B, COUT, H, W), np.float32)
    for c in range(NCORES):
        r = res.results[c]["out"]
        out[0, BL * c : BL * c + BL] = r[:COUT].transpose(1, 0, 2, 3)
        out[1, BL * c : BL * c + BL] = r[COUT:].transpose(1, 0, 2, 3)
    if _trace:
        _CACHE["last_result"] = res
    return out


# revision 15
# speedup vs baseline: 1.2658x; 1.2658x over previous
"""Complex CNN 2d (conv + complex-combine + training-mode BatchNorm) on 8 trn2 cores.

Strategy (hardcoded for B=32, Cin=2, Cout=64, H=W=128, K=5, pad=2, stride=1):
  - Data-parallel over batch: 4 images per core.
  - Conv as matmul.  Contract dim = (plane, ky, kxo) = 4*5*3 = 60 rows per
    image (kx = kxo + 3*kxi with kxo pre-shifted into the SBUF layout), so a
    PSUM bank accumulates in just S=2 matmul steps (kxi = 0,1; the kx=5
    combination is zero-weighted).  fp32r streaming runs at 1 col/cycle but
    monopolizes the XBUS fabric, so fewer accumulation steps - not row-group
    concurrency - is what cuts PE time.
  - Residency: partition block 0 (rows 0..63) holds images 0 and 2 (two
    free-dim halves), block 1 (rows 64..127) holds images 1 and 3.  Rows
    60..63 of each block are zero (and zero-weighted) to pad the contract to
    64.  Everything stays in SBUF across both passes.
  - Out channels = 128 = [64 real | 64 imag]; the complex combine is folded
    into the weight matrix signs.
  - Exact global BN stats: pass 1 computes conv + per-core (mean, E[Y^2]) via
    bn_stats/bn_aggr on DVE, AllReduce over the 8 cores, then pass 2
    recomputes conv and applies y*scale+shift on the Scalar engine (ACT),
    which keeps DVE free for bn_stats.  Y never round-trips to HBM.
  - Conv bias br/bi provably cancels in BN (shifts mean equally) -> ignored.
"""

import sys

sys.path.insert(0, "/opt/trn_rl_repo")

import numpy as np

B, CIN, COUT, H, W, K, PAD = 32, 2, 64, 128, 128, 5, 2
EPS = 1e-5
NCORES = 8
BL = B // NCORES  # 4 local images per core
NPLANES = 2 * CIN  # r0, r1, i0, i1
RKX = 3  # kx replication factor (kxo = 0..2)
SKX = 2  # accumulation steps (kxi = 0..1)
KROWS = NPLANES * K * RKX  # 60 contract rows per image
KPAD = 64  # padded contract rows (rows 60..63 zero)
WPAD = W + 4  # 132 cols per stored plane row
PLANE = H * WPAD  # elements per stored plane
CTOT = 2 * COUT  # 128 fused out channels: [real 64 | imag 64]
YB = 4  # y-rows per PSUM bank (4*128 = 512 = one fp32 bank)
NBLK = H // YB  # 32 blocks
MM_DT = "float32r"  # matmul streaming dtype (full-rate fp32 path)

ZWLEN = 2 * PLANE + SKX * CTOT  # per-partition: 2 image halves + weights

_CACHE = {}


def _build_nc():
    import concourse.tile as tile
    from concourse import bacc, mybir

    f32 = mybir.dt.float32
    mdt = getattr(mybir.dt, MM_DT)

    # Bacc (not plain Bass): its compile pipeline splits multi-sem waits into
    # event-semaphore preludes, which TRN2 instruction structs require
    nc = bacc.Bacc(num_devices=NCORES)
    z_d = nc.dram_tensor("zw", [128, ZWLEN], mdt, kind="ExternalInput")
    g_d = nc.dram_tensor("gamma", [CTOT, 1], f32, kind="ExternalInput")
    bt_d = nc.dram_tensor("beta", [CTOT, 1], f32, kind="ExternalInput")
    o_d = nc.dram_tensor("out", [CTOT, BL, H, W], f32, kind="ExternalOutput")

    with tile.TileContext(nc) as tc:
        with (
            tc.tile_pool(name="const", bufs=1) as const,
            tc.tile_pool(name="psum", bufs=1, space="PSUM") as psum,
            tc.tile_pool(name="outp", bufs=8) as outp,
            tc.tile_pool(name="small", bufs=1) as small,
            tc.tile_pool(name="dram", bufs=1, space="DRAM") as dram,
        ):
            zw = const.tile([128, ZWLEN], mdt)
            # one DMA per 64-partition contract block so each matmul waits on
            # one sem (a fused-LDW fp32r matmul supports only one sync wait)
            for c in range(2):
                nc.sync.dma_start(
                    out=zw[64 * c : 64 * c + 64], in_=z_d[64 * c : 64 * c + 64]
                )
            # image-half views: [partition, y, x]
            zv = [
                zw[:, half * PLANE : (half + 1) * PLANE].rearrange(
                    "p (h w) -> p h w", h=H
                )
                for half in range(2)
            ]
            wt = zw[:, 2 * PLANE :].rearrange("p (s c) -> p s c", s=SKX)
            gt = const.tile([CTOT, 1], f32)
            nc.sync.dma_start(out=gt[:], in_=g_d[:])
            bt = const.tile([CTOT, 1], f32)
            nc.sync.dma_start(out=bt[:], in_=bt_d[:])
            eps_t = const.tile([CTOT, 1], f32)
            nc.vector.memset(eps_t[:], EPS)

            # 8 persistent PSUM bank tiles (all 8 banks): same tensors across
            # all blocks, so bank-WAW between matmuls is same-tensor and each
            # first matmul carries at most the single WAR wait on the previous
            # consumer.
            pbanks = [
                psum.tile([CTOT, YB, W], f32, name=f"pbank{i}", tag=f"pbank{i}", bufs=1)
                for i in range(2 * BL)
            ]

            def conv_block(blk, consume):
                ys = blk * YB
                banks = pbanks[BL * (blk % 2) : BL * (blk % 2) + BL]
                for kxi in range(SKX):
                    for b in range(BL):
                        base = 64 * (b % 2)
                        half = b // 2
                        nc.tensor.matmul(
                            banks[b][:, :, :],
                            wt[base : base + KPAD, kxi, :],
                            zv[half][
                                base : base + KPAD,
                                ys : ys + YB,
                                RKX * kxi : RKX * kxi + W,
                            ],
                            start=(kxi == 0),
                            stop=(kxi == SKX - 1),
                        )
                for b in range(BL):
                    consume(b, banks[b], ys)

            # ---- pass 1: conv + per-core stats ----
            stats = small.tile([CTOT, NBLK * BL, 6], f32)

            def stat_consume(b, bank, ys):
                blk = ys // YB
                e = blk * BL + b
                nc.vector.bn_stats(
                    out=stats[:, e, :],
                    in_=bank[:, :, :].rearrange("p a b -> p (a b)"),
                )

            for blk in range(NBLK):
                conv_block(blk, stat_consume)

            mv = small.tile([CTOT, 2], f32)
            nc.vector.bn_aggr(out=mv[:], in_=stats[:])
            # pack (mean, E[Y^2]) for the cross-core all-reduce
            pair = small.tile([CTOT, 2], f32)
            nc.vector.tensor_copy(out=pair[:, 0:1], in_=mv[:, 0:1])
            msq = small.tile([CTOT, 1], f32)
            nc.vector.tensor_mul(out=msq[:], in0=mv[:, 0:1], in1=mv[:, 0:1])
            nc.vector.tensor_add(out=pair[:, 1:2], in0=mv[:, 1:2], in1=msq[:])

            cc_in = dram.tile([CTOT, 2], f32)
            cc_out = dram.tile([CTOT, 2], f32)
            nc.gpsimd.dma_start(out=cc_in[:], in_=pair[:])
            nc.gpsimd.collective_compute(
                "AllReduce",
                mybir.AluOpType.add,
                replica_groups=[list(range(NCORES))],
                ins=[cc_in[:].opt()],
                outs=[cc_out[:].opt()],
            )
            red = small.tile([CTOT, 2], f32)
            nc.gpsimd.dma_start(out=red[:], in_=cc_out[:])

            # global mean / var -> scale, shift
            mean_g = small.tile([CTOT, 1], f32)
            nc.vector.tensor_scalar_mul(
                out=mean_g[:], in0=red[:, 0:1], scalar1=1.0 / NCORES
            )
            ey2_g = small.tile([CTOT, 1], f32)
            nc.vector.tensor_scalar_mul(
                out=ey2_g[:], in0=red[:, 1:2], scalar1=1.0 / NCORES
            )
            mg2 = small.tile([CTOT, 1], f32)
            nc.vector.tensor_mul(out=mg2[:], in0=mean_g[:], in1=mean_g[:])
            var_g = small.tile([CTOT, 1], f32)
            nc.vector.tensor_sub(out=var_g[:], in0=ey2_g[:], in1=mg2[:])
            std = small.tile([CTOT, 1], f32)
            nc.scalar.activation(
                out=std[:], in_=var_g[:],
                func=mybir.ActivationFunctionType.Sqrt,
                bias=eps_t[:], scale=1.0,
            )
            rstd = small.tile([CTOT, 1], f32)
            nc.vector.reciprocal(out=rstd[:], in_=std[:])
            scale_t = small.tile([CTOT, 1], f32)
            nc.vector.tensor_mul(out=scale_t[:], in0=gt[:], in1=rstd[:])
            mscale = small.tile([CTOT, 1], f32)
            nc.vector.tensor_mul(out=mscale[:], in0=mean_g[:], in1=scale_t[:])
            shift_t = small.tile([CTOT, 1], f32)
            nc.vector.tensor_sub(out=shift_t[:], in0=bt[:], in1=mscale[:])

            # ---- pass 2: conv again + affine apply (ACT) + store ----
            def apply_consume(b, bank, ys):
                ob = outp.tile([CTOT, YB, W], f32, tag="ob", name=f"ob{ys}_{b}")
                nc.scalar.activation(
                    out=ob[:], in_=bank[:, :, :],
                    func=mybir.ActivationFunctionType.Identity,
                    bias=shift_t[:], scale=scale_t[:],
                )
                nc.sync.dma_start(out=o_d[:, b, ys : ys + YB, :], in_=ob[:])

            for blk in range(NBLK):
                conv_block(blk, apply_consume)

    nc.finalize()
    return nc


def _get_nc():
    if "nc" not in _CACHE:
        _CACHE["nc"] = _build_nc()
    return _CACHE["nc"]


def _pack_inputs(Xr, Xi, Wr, Wi, gamma_r, beta_r, gamma_i, beta_i):
    planes = np.stack([Xr[:, 0], Xr[:, 1], Xi[:, 0], Xi[:, 1]], axis=1)  # [B,4,H,W]
    planes = np.ascontiguousarray(planes, dtype=np.float32)

    ZW = np.zeros((NCORES, 128, ZWLEN), np.float32)
    zw_img = ZW[:, :, : 2 * PLANE].reshape(NCORES, 128, 2, H, WPAD)
    for ky in range(K):
        r0, r1 = max(0, PAD - ky), min(H, H + PAD - ky)
        s0, s1 = r0 + ky - PAD, r1 + ky - PAD
        for kxo in range(RKX):
            c0, c1 = max(0, PAD - kxo), min(WPAD, W + PAD - kxo)
            d0, d1 = c0 + kxo - PAD, c1 + kxo - PAD
            for pi in range(NPLANES):
                q = pi * (K * RKX) + ky * RKX + kxo
                for b in range(BL):
                    part = 64 * (b % 2) + q
                    half = b // 2
                    for c in range(NCORES):
                        zw_img[c, part, half, r0:r1, c0:c1] = planes[
                            BL * c + b, pi, s0:s1, d0:d1
                        ]

    # weights: [partition, kxi, outch];  kx = kxo + 3*kxi, zero for kx > 4
    Wf = np.zeros((KPAD, SKX, CTOT), np.float32)
    for pi in range(NPLANES):
        for ky in range(K):
            for kxo in range(RKX):
                q = pi * (K * RKX) + ky * RKX + kxo
                for kxi in range(SKX):
                    kx = kxo + RKX * kxi
                    if kx >= K:
                        continue
                    if pi < 2:
                        Wf[q, kxi, :COUT] = Wr[:, pi, ky, kx]
                        Wf[q, kxi, COUT:] = Wi[:, pi, ky, kx]
                    else:
                        Wf[q, kxi, :COUT] = -Wi[:, pi - 2, ky, kx]
                        Wf[q, kxi, COUT:] = Wr[:, pi - 2, ky, kx]
    wrep = np.zeros((128, SKX, CTOT), np.float32)
    wrep[:KPAD] = Wf
    wrep[64 : 64 + KPAD] = Wf
    ZW[:, :, 2 * PLANE :] = wrep.reshape(128, SKX * CTOT)[None]

    gam = np.concatenate([gamma_r, gamma_i]).astype(np.float32).reshape(CTOT, 1)
    bet = np.concatenate([beta_r, beta_i]).astype(np.float32).reshape(CTOT, 1)

    return [{"zw": ZW[c], "gamma": gam, "beta": bet} for c in range(NCORES)]


def _run(in_maps, trace=False):
    from concourse.bass_utils import run_bass_kernel_spmd

    nc = _get_nc()
    return run_bass_kernel_spmd(nc, in_maps, list(range(NCORES)), trace=trace)


def kernel(Xr, Xi, Wr, Wi, br, bi, gamma_r, beta_r, gamma_i, beta_i, _trace=False):
    Xr = np.asarray(Xr, np.float32)
    Xi = np.asarray(Xi, np.float32)
    Wr = np.asarray(Wr, np.float32)
    Wi = np.asarray(Wi, np.float32)
    in_maps = _pack_inputs(
        Xr, Xi, Wr, Wi,
        np.asarray(gamma_r), np.asarray(beta_r),
        np.asarray(gamma_i), np.asarray(beta_i),
    )
    res = _run(in_maps, trace=_trace)
    out = np.empty((2, B, COUT, H, W), np.float32)
    for c in range(NCORES):
        r = res.results[c]["out"]
        out[0, BL * c : BL * c + BL] = r[:COUT].transpose(1, 0, 2, 3)
        out[1, BL * c : BL * c + BL] = r[COUT:].transpose(1, 0, 2, 3)
    if _trace:
        _CACHE["last_result"] = res
    return out
